# revision 11
# baseline (speedup 1.0000x reference)
"""DMoN graph-pooling kernel for 8 Trainium2 NeuronCores.

Math reformulation (no scatter needed):
  S   = softmax(X @ W.T + b)                      [N, k]
  cs  = S.T @ 1                                   [k]      (cluster_sizes)
  M   = S.T @ X                                   [k, D]
  T   = sum_e w_e * S[c_e] (x) S[r_e]             [k, k]   (= (A@S).T @ S reordered)
  v   = sum_e w_e * S[c_e]                        [k]      (= S.T @ degrees)
  E   = sum_e w_e
  trace(graph_pooled)  = trace(T)
  trace(normalizer)    = (v . v) / (2E)
  spectral_loss        = -(trace(T) - (v.v)/(2E)) / (2E)
  collapse_loss        = 0.1 * (||cs|| / n * sqrt(k) - 1)
  features_pooled      = selu(M / cs[:, None])

Distribution: all 8 cores replicate the softmax pass (each needs the full S
in its own HBM for edge gathers), but each core only accumulates cs/M and
writes the f32 S output for its own 1/8 node slice, and processes its own
1/8 edge shard.  SPMD divergence is avoided by rotating the node array per
core (host-side np.roll) so that "my slice" is always rows [0, SLICE) —
edge endpoint indices are remapped to the rotated coordinates per core.
Final tiny combines (sums of [64,65]-sized partials, selu, losses) happen
on the host.
"""

import numpy as np


def _ensure_paths():
    import sys
    try:
        import concourse  # noqa: F401
        return
    except ImportError:
        pass
    for p in ("/opt/trn_rl_repo", "/root/.axon_site/_ro/trn_rl_repo"):
        if p not in sys.path:
            sys.path.insert(0, p)
    import concourse  # noqa: F401


# ---------------------------------------------------------------- config ---

class CFG:
    """Full-size problem configuration."""
    N_NODES = 100000
    D = 128
    K = 64
    N_EDGES = 3200000
    N_CORES = 8

    NB = 7                # node chunks (of 128 rows) per phase-1 batch
    SLICE_BATCHES = 14    # batches per core slice
    ET_COLS = 128         # gather offsets per partition per edge tile
    NTILES = 25           # edge tiles per core

    @classmethod
    def derived(cls):
        cls.SLICE_CHUNKS = cls.NB * cls.SLICE_BATCHES          # 98
        cls.SLICE = cls.SLICE_CHUNKS * 128                     # 12544
        cls.NPAD = cls.SLICE * cls.N_CORES                     # 100352
        cls.NBATCH = cls.NPAD // (cls.NB * 128)                # 112
        cls.EPC = cls.N_EDGES // cls.N_CORES                   # 400000
        cls.ET_EDGES = 128 * cls.ET_COLS                       # 16384
        cls.EPAD = cls.NTILES * cls.ET_EDGES                   # 409600
        assert cls.EPAD >= cls.EPC
        assert cls.NPAD >= cls.N_NODES
        return cls


CFG.derived()


def _split_excess_waits(nc, max_waits=1):
    """walrus's CoreV3 codegen rejects instructions carrying more than ~2
    semaphore waits ("Too many sync wait commands").  Move excess waits onto
    dedicated same-engine NoOps inserted immediately before the instruction
    (engine FIFO order preserves the semantics)."""
    from concourse import mybir

    ctr = 0
    for fn in nc.m.functions:
        for bb in fn.blocks:
            new = []
            for ins in bb.instructions:
                si = ins.sync_info
                waits = list(si.on_wait) if si is not None and (si.on_wait or []) else []
                if len(waits) > max_waits:
                    extra, keep = waits[:-max_waits], waits[-max_waits:]
                    for w in extra:
                        n = mybir.InstNoOp(name=f"I-wsplit-{ctr}", ins=[], outs=[])
                        ctr += 1
                        n.engine = ins.engine
                        n.sync_info = mybir.SyncInfo(on_wait=[w], on_update=[])
                        new.append(n)
                    ins.sync_info = mybir.SyncInfo(
                        on_wait=keep, on_update=list(si.on_update or []))
                new.append(ins)
            bb.instructions = new


# ---------------------------------------------------------------- kernel ---

def build_nc(cfg=CFG):
    _ensure_paths()
    import concourse.bass as bass
    import concourse.tile as tile
    from concourse import mybir
    from concourse.masks import make_identity

    f32 = mybir.dt.float32
    f16 = mybir.dt.float16
    i32 = mybir.dt.int32
    AX = mybir.AxisListType
    OP = mybir.AluOpType
    AF = mybir.ActivationFunctionType

    NB, K, D = cfg.NB, cfg.K, cfg.D
    NPAD, NBATCH, SLICE_BATCHES = cfg.NPAD, cfg.NBATCH, cfg.SLICE_BATCHES
    SLICE, SLICE_CHUNKS = cfg.SLICE, cfg.SLICE_CHUNKS
    ET_COLS, NTILES, EPAD = cfg.ET_COLS, cfg.NTILES, cfg.EPAD

    nc = bass.Bass()

    x_in = nc.dram_tensor("features", [NPAD, D], f32, kind="ExternalInput")
    wt_in = nc.dram_tensor("w_t", [D, K], f32, kind="ExternalInput")
    mask_in = nc.dram_tensor("mask", [128, SLICE_CHUNKS], f32, kind="ExternalInput")
    er_in = nc.dram_tensor("edge_rows", [EPAD], i32, kind="ExternalInput")
    ec_in = nc.dram_tensor("edge_cols", [EPAD], i32, kind="ExternalInput")
    ew_in = nc.dram_tensor("edge_vals", [EPAD], f32, kind="ExternalInput")

    s_out = nc.dram_tensor("s_out", [SLICE, K], f32, kind="ExternalOutput")
    m_out = nc.dram_tensor("m_out", [K, D], f32, kind="ExternalOutput")
    cs_out = nc.dram_tensor("cs_out", [K, 1], f32, kind="ExternalOutput")
    t_out = nc.dram_tensor("t_out", [K, K + 1], f32, kind="ExternalOutput")

    # fp16 copy of S used by the edge-phase gathers; one spare row because
    # the row-side gather reads 65 elements per index (the 65th lands in the
    # ones column and is overwritten, but must not read out of bounds).
    s16 = nc.dram_tensor("s16", [NPAD + 1, K], f16)

    # batched DRAM views: batch t, partition p, chunk j, feature d
    x_v = x_in[:].rearrange("(t j p) d -> t p j d", p=128, j=NB)
    s16_v = s16[0:NPAD, :].rearrange("(t j p) k -> t p j k", p=128, j=NB)
    sout_v = s_out[:].rearrange("(t j p) k -> t p j k", p=128, j=NB)
    er_v = er_in[:].rearrange("(t p j) -> t p j", p=128, j=ET_COLS)
    ec_v = ec_in[:].rearrange("(t p j) -> t p j", p=128, j=ET_COLS)
    ew_v = ew_in[:].rearrange("(t p j) -> t p j", p=128, j=ET_COLS)

    with tile.TileContext(nc) as tc:
        with (
            tc.tile_pool(name="const", bufs=1) as cpool,
            tc.tile_pool(name="ph1", bufs=2) as pool1,
            tc.tile_pool(name="ph1ps", bufs=2, space="PSUM") as ppool1,
            tc.tile_pool(name="accps", bufs=1, space="PSUM") as apool,
        ):
            identity = cpool.tile([128, 128], f32)
            make_identity(nc, identity[:])
            wt_sb = cpool.tile([D, K], f32)
            nc.sync.dma_start(out=wt_sb[:], in_=wt_in[:])
            mask_sb = cpool.tile([128, SLICE_CHUNKS], f32)
            nc.sync.dma_start(out=mask_sb[:], in_=mask_in[:])
            # zero the spare overread row of s16
            z16 = cpool.tile([1, K], f16)
            nc.vector.memset(z16[:], 0.0)
            nc.sync.dma_start(out=s16[NPAD:NPAD + 1, :], in_=z16[:])

            m_ps = apool.tile([K, D], f32, space="PSUM")
            cs_ps = apool.tile([K, 1], f32, space="PSUM")

            # ---------------- phase 1: softmax assignments -----------------
            for t in range(NBATCH):
                in_slice = t < SLICE_BATCHES
                xb = pool1.tile([128, NB, D], f32, tag="xb")
                nc.sync.dma_start(out=xb[:], in_=x_v[t])

                xt_ps = ppool1.tile([128, NB, 128], f32, space="PSUM", tag="xt")
                for j in range(NB):
                    nc.tensor.transpose(xt_ps[:, j, :], xb[:, j, :], identity[:])
                xt_sb = pool1.tile([128, NB, 128], f32, tag="xt_sb")
                nc.vector.tensor_copy(xt_sb[:], xt_ps[:])

                lg_ps = ppool1.tile([128, NB, K], f32, space="PSUM", tag="lg")
                for j in range(NB):
                    nc.tensor.matmul(lg_ps[:, j, :], xt_sb[:, j, :], wt_sb[:],
                                     start=True, stop=True)

                ex = pool1.tile([128, NB, K], f32, tag="ex")
                nc.scalar.activation(ex[:], lg_ps[:], AF.Exp)
                sums = pool1.tile([128, NB], f32, tag="sums")
                nc.vector.tensor_reduce(sums[:], ex[:], axis=AX.X, op=OP.add)
                rec = pool1.tile([128, NB], f32, tag="rec")
                nc.vector.reciprocal(rec[:], sums[:])

                s16b = pool1.tile([128, NB, K], f16, tag="s16b")
                if in_slice:
                    s32b = pool1.tile([128, NB, K], f32, tag="s32b")
                    nc.vector.tensor_tensor(
                        out=s32b[:], in0=ex[:],
                        in1=rec[:].to_broadcast([128, NB, K]), op=OP.mult)
                    nc.vector.tensor_copy(s16b[:], s32b[:])
                    nc.sync.dma_start(out=sout_v[t], in_=s32b[:])
                    for j in range(NB):
                        c = t * NB + j
                        nc.tensor.matmul(m_ps[:], s32b[:, j, :], xb[:, j, :],
                                         start=(c == 0), stop=(c == SLICE_CHUNKS - 1))
                        nc.tensor.matmul(cs_ps[:], s32b[:, j, :],
                                         mask_sb[:, c:c + 1],
                                         start=(c == 0), stop=(c == SLICE_CHUNKS - 1))
                else:
                    nc.vector.tensor_tensor(
                        out=s16b[:], in0=ex[:],
                        in1=rec[:].to_broadcast([128, NB, K]), op=OP.mult)
                nc.sync.dma_start(out=s16_v[t], in_=s16b[:])

            m_sb = pool1.tile([K, D], f32, tag="m_sb")
            nc.vector.tensor_copy(m_sb[:], m_ps[:])
            nc.sync.dma_start(out=m_out[:], in_=m_sb[:])
            cs_sb = pool1.tile([K, 1], f32, tag="cs_sb")
            nc.vector.tensor_copy(cs_sb[:], cs_ps[:])
            nc.sync.dma_start(out=cs_out[:], in_=cs_sb[:])

        # ------------------- phase 2: edge contraction ---------------------
        with (
            tc.tile_pool(name="ph2", bufs=2) as pool2,
            tc.tile_pool(name="ph2ps", bufs=1, space="PSUM") as ppool2,
        ):
            t_ps = ppool2.tile([K, K + 1], f32, space="PSUM")
            for ti in range(NTILES):
                idc = pool2.tile([128, ET_COLS], i32, tag="idc")
                nc.sync.dma_start(out=idc[:], in_=ec_v[ti])
                idr = pool2.tile([128, ET_COLS], i32, tag="idr")
                nc.sync.dma_start(out=idr[:], in_=er_v[ti])
                wv = pool2.tile([128, ET_COLS], f32, tag="wv")
                nc.sync.dma_start(out=wv[:], in_=ew_v[ti])
                wv16 = pool2.tile([128, ET_COLS], f16, tag="wv16")
                nc.vector.tensor_copy(wv16[:], wv[:])

                gc = pool2.tile([128, ET_COLS, K], f16, tag="gc")
                nc.gpsimd.indirect_dma_start(
                    out=gc[:], out_offset=None, in_=s16[:],
                    in_offset=bass.IndirectOffsetOnAxis(ap=idc[:], axis=0))
                gr = pool2.tile([128, ET_COLS, K + 1], f16, tag="gr")
                nc.gpsimd.indirect_dma_start(
                    out=gr[:], out_offset=None, in_=s16[:],
                    in_offset=bass.IndirectOffsetOnAxis(ap=idr[:], axis=0))
                # ones column for the v / E accumulation
                nc.vector.memset(gr[:, :, K:K + 1], 1.0)

                sc = pool2.tile([128, ET_COLS, K], f16, tag="sc")
                nc.vector.tensor_tensor(
                    out=sc[:], in0=gc[:],
                    in1=wv16[:].to_broadcast([128, ET_COLS, K]), op=OP.mult)

                for j in range(ET_COLS):
                    nc.tensor.matmul(t_ps[:], sc[:, j, :], gr[:, j, :],
                                     start=(ti == 0 and j == 0),
                                     stop=(ti == NTILES - 1 and j == ET_COLS - 1))

            t_sb = pool2.tile([K, K + 1], f32, tag="t_sb")
            nc.vector.tensor_copy(t_sb[:], t_ps[:])
            nc.sync.dma_start(out=t_out[:], in_=t_sb[:])

    _split_excess_waits(nc, max_waits=1)
    return nc


_NC_CACHE = {}


def _get_nc(cfg=CFG):
    key = (cfg.NPAD, cfg.EPAD, cfg.NB, cfg.ET_COLS)
    if key not in _NC_CACHE:
        _NC_CACHE[key] = build_nc(cfg)
    return _NC_CACHE[key]


def make_in_maps(features, W, b, edge_vals, edge_rows, edge_cols, cfg=CFG):
    n, d = features.shape
    k = W.shape[0]
    assert (n, d, k) == (cfg.N_NODES, cfg.D, cfg.K)

    x_pad = np.zeros((cfg.NPAD, cfg.D), dtype=np.float32)
    x_pad[:n] = np.asarray(features, dtype=np.float32)
    w_t = np.ascontiguousarray(np.asarray(W, dtype=np.float32).T)

    ev = np.asarray(edge_vals, dtype=np.float32)
    er = np.asarray(edge_rows, dtype=np.int64)
    ec = np.asarray(edge_cols, dtype=np.int64)

    in_maps = []
    for core in range(cfg.N_CORES):
        shift = cfg.SLICE * core
        x_rot = np.roll(x_pad, -shift, axis=0) if shift else x_pad.copy()

        # per-core slice validity mask [p, c] for global row shift + c*128 + p
        p = np.arange(128)[:, None]
        c = np.arange(cfg.SLICE_CHUNKS)[None, :]
        gl = shift + c * 128 + p
        mask = (gl < n).astype(np.float32)

        lo, hi = core * cfg.EPC, (core + 1) * cfg.EPC
        er_k = np.zeros(cfg.EPAD, dtype=np.int32)
        ec_k = np.zeros(cfg.EPAD, dtype=np.int32)
        ew_k = np.zeros(cfg.EPAD, dtype=np.float32)
        cnt = hi - lo
        er_k[:cnt] = ((er[lo:hi] - shift) % cfg.NPAD).astype(np.int32)
        ec_k[:cnt] = ((ec[lo:hi] - shift) % cfg.NPAD).astype(np.int32)
        ew_k[:cnt] = ev[lo:hi]

        in_maps.append({
            "features": x_rot,
            "w_t": w_t,
            "mask": np.ascontiguousarray(mask),
            "edge_rows": er_k,
            "edge_cols": ec_k,
            "edge_vals": ew_k,
        })
    return in_maps


def combine_outputs(results, edge_vals, cfg=CFG):
    """results: list of per-core output dicts."""
    n, k = cfg.N_NODES, cfg.K
    s_pad = np.concatenate([results[c]["s_out"] for c in range(cfg.N_CORES)], axis=0)
    assignments = np.ascontiguousarray(s_pad[:n]).astype(np.float32)

    cs = np.sum([results[c]["cs_out"][:, 0] for c in range(cfg.N_CORES)],
                axis=0, dtype=np.float64)
    m = np.sum([results[c]["m_out"] for c in range(cfg.N_CORES)],
               axis=0, dtype=np.float64)
    t_ext = np.sum([results[c]["t_out"] for c in range(cfg.N_CORES)],
                   axis=0, dtype=np.float64)
    t_mat = t_ext[:, :k]
    v = t_ext[:, k]

    e_tot = float(np.sum(np.asarray(edge_vals, dtype=np.float64)))
    t1 = float(np.trace(t_mat))
    vv = float(np.dot(v, v))
    spectral_loss = np.float32(-(t1 - vv / (2.0 * e_tot)) / (2.0 * e_tot))

    collapse_loss = np.float32(
        0.1 * (np.linalg.norm(cs) / n * np.sqrt(k) - 1.0))

    # features_pooled = selu(M / cs[:, None])  (jax.nn.selu constants)
    scale = 1.0507009873554805
    alpha = 1.6732632423543772
    x = (m / cs[:, None]).astype(np.float32)
    features_pooled = np.where(
        x > 0, scale * x, np.float32(scale * alpha) * np.expm1(x)
    ).astype(np.float32)

    return features_pooled, assignments, spectral_loss, collapse_loss


_last_results = None


def _numpy_fallback(features, W, b, edge_vals, edge_rows, edge_cols):
    """Exact numpy implementation; only used if b != 0 (never in practice —
    the device kernel folds softmax without the always-zero bias)."""
    n, d = features.shape
    k = W.shape[0]
    logits = features @ W.T + b
    e = np.exp(logits - logits.max(axis=1, keepdims=True))
    S = (e / e.sum(axis=1, keepdims=True)).astype(np.float32)
    cs = S.sum(axis=0, dtype=np.float64)
    deg = np.zeros(n); np.add.at(deg, edge_cols, edge_vals.astype(np.float64))
    AS = np.zeros((n, k))
    np.add.at(AS, edge_rows, edge_vals[:, None].astype(np.float64) * S[edge_cols])
    gp = AS.T @ S
    e2 = 2 * deg.sum()
    nl = S.T.astype(np.float64) @ deg
    spectral = np.float32(-(np.trace(gp) - np.dot(nl, nl) / e2) / e2)
    collapse = np.float32(0.1 * (np.linalg.norm(cs) / n * np.sqrt(k) - 1.0))
    M = S.T.astype(np.float64) @ features
    x = (M / cs[:, None]).astype(np.float32)
    scale, alpha = 1.0507009873554805, 1.6732632423543772
    fp = np.where(x > 0, scale * x,
                  np.float32(scale * alpha) * np.expm1(x)).astype(np.float32)
    return fp, S, spectral, collapse


def kernel(features, W, b, edge_vals, edge_rows, edge_cols):
    global _last_results
    if np.any(np.asarray(b) != 0):
        return _numpy_fallback(np.asarray(features), np.asarray(W),
                               np.asarray(b), np.asarray(edge_vals),
                               np.asarray(edge_rows), np.asarray(edge_cols))
    _ensure_paths()
    from concourse.bass_utils import run_bass_kernel_spmd

    cfg = CFG
    nc = _get_nc(cfg)
    in_maps = make_in_maps(features, W, b, edge_vals, edge_rows, edge_cols, cfg)
    res = run_bass_kernel_spmd(nc, in_maps, core_ids=list(range(cfg.N_CORES)))
    _last_results = res
    return combine_outputs(res.results, edge_vals, cfg)
